# revision 7
# baseline (speedup 1.0000x reference)
"""TRN2 Bass kernel for nn_Attention_70257075028315.

reference:
    scores = einsum('bqd,bkd->bqk', query, key)       # B=8, Nq=Nk=2048, D=512
    probs  = softmax(scores, -1)
    out    = einsum('bqk,bkd->bqd', probs, key)

Sharding: batch b -> NeuronCore b (data parallel, fully local attention).

v2 design (per-core q/k: [2048, 512] fp32). Everything 16-bit on the PE,
p-transposes on the DMA XBAR, all loads cast in-flight by the software DGE:

  Load: per 4-tile group, a gpsimd (swdge) casting DMA moves fp32 DRAM ->
    fp16 SBUF natural layout (k16/q16, k16 doubles as the PV rhs); fp16 PE
    transposes (1 cyc/row; PE is idle during load) + DVE PSUM->SBUF copies
    build kT/qT [128(d%128), dchunk, tile, 128] (contiguous S rhs slices).
    Cast order K g0, Q g0, K g1-3; K group-g transposes are interleaved
    one chunk ahead into tile 0's S chunks (tile 0 runs c-order so chunk c
    needs only K group c), so S(0) isn't queued behind transposes waiting
    on the last cast; tile 0 borrows the idle PV PSUM banks for two chunks
    so tiles 1-2 don't stall on S-bank reuse while the pipeline fills. A
    dummy exp at t=0 preloads the ACT exp table (saves 1.3us on first exp).
  Phase C (per q-tile, software-pipelined with lookahead=2):
    S     = qT.T @ kT   fp16, 4 d-chunk-accumulated matmuls per 512-wide
            chunk, each chunk in its own PSUM bank; dc-outer order keeps
            lhsT constant over 4 consecutive matmuls (fewer weight loads)
    max   per chunk on DVE as each chunk completes; combined, negated
    p     = exp(S - max) fp16: one ACT pass per chunk PSUM -> SBUF with
            fused row-sum accum; 1/sum via DVE reciprocal
    pT    = per-chunk XBAR dma-transposes ([128,512]->[128,4,128] fp16,
            SP queue) fired as each exp lands -- no PE transposes and no
            PSUM round-trip for pT
    o     = pT.T @ k16  fp16, 16 kk-accumulated matmuls -> PSUM [128, 512]
    out   = o * (1/rowsum) on DVE/ACT (alternating), out-DMA on the Pool
            (swdge) queue to keep SP free for the p XBARs (last tile via
            sync HWDGE for a shorter drain).
  Emission per step i: S(i+2)+E(i+2), PV(i), with a PE-queue dep keeping
  PV(i) after S(i+2) so PV hides tile i+2's max->exp->xbar latency chain
  (~6us, needs the 2-tile lookahead). PSUM: 4 S banks + 2 PV + 2 load-tr.

Dtype: fp16 everywhere on the PE. HW-measured: 16-bit matmuls stream
1 col/cycle (same as f32r >=256 wide, 2x faster than fp32), but fp16
operands halve SBUF traffic and let kT/qT/pT/k16 fit in 44KB/partition.
Score error vs f32r only ~2x (fp16 keeps 10 mantissa bits); measured
output rel err 1.5e-3 (budget 2e-2). bf16 scores would be 8x worse and
fp8 fails the budget outright.
"""

import numpy as np

import concourse.bass as bass
import concourse.tile as tile
import concourse.mybir as mybir
from concourse import bacc
from concourse.bass_utils import run_bass_kernel_spmd

FP32 = mybir.dt.float32
FP32R = mybir.dt.float32r
FP16 = mybir.dt.float16
AF = mybir.ActivationFunctionType

B, NQ, NK, D = 8, 2048, 2048, 512
P = 128
NKT = NK // P   # 16 kk tiles
NQT = NQ // P   # 16 q tiles
NDC = D // P    # 4 d chunks
NCH = NK // 512  # 4 score chunks of 512


def build(score_dtype=FP16, repeat_c=1, timed=False, pv_dtype=FP16,
          swdge_cast=True, xbar_splits=4, scale_eng="alt", lookahead=2,
          load_mode="pe", s_order="dc", q_lazy=None, probe=None,
          dedup_ldw=True, kpv_bf16=False, timed_scope="c"):
    """timed=True adds an int32 [1,1] input "reps": phase C re-runs in a
    dynamic For_i loop `reps` more times (0 = just the normal kernel), so one
    NEFF can measure the phase-C slope against itself."""
    nc = bacc.Bacc("TRN2", target_bir_lowering=False, debug=False)
    q_d = nc.dram_tensor("query", [NQ, D], FP32, kind="ExternalInput").ap()
    k_d = nc.dram_tensor("key", [NK, D], FP32, kind="ExternalInput").ap()
    reps_d = None
    if timed:
        reps_d = nc.dram_tensor(
            "reps", [1, 1], mybir.dt.int32, kind="ExternalInput"
        ).ap()
    out_d = nc.dram_tensor("out", [NQ, D], FP32, kind="ExternalOutput").ap()

    q_tiles_d = q_d.rearrange("(t p) d -> t p d", p=P)
    k_tiles_d = k_d.rearrange("(t p) d -> t p d", p=P)
    out_tiles_d = out_d.rearrange("(t p) d -> t p d", p=P)

    with tile.TileContext(nc) as tc:
        _body(tc, q_tiles_d, k_tiles_d, out_tiles_d, score_dtype, repeat_c,
              reps_d, pv_dtype, swdge_cast, xbar_splits, scale_eng, lookahead,
              load_mode, s_order, q_lazy, probe, timed_scope)
    nc.compile()
    if dedup_ldw:
        # NOTE: do NOT reorder instructions here (e.g. regrouping S matmuls
        # by weights): compile() lowers waits to per-engine completion-count
        # semaphores, so same-engine reordering breaks wait semantics (the
        # race detector catches it). Removing a non-updating instruction and
        # merging its waits forward is the only safe post-compile transform.
        _dedup_ldweights(nc)
    return nc


def _dedup_ldweights(nc):
    """Drop InstLdweights that reload the exact weights already in the PE
    array (dc-outer S keeps lhsT constant across 4 consecutive matmuls, but
    the lowering emits one Ldweights per matmul and walrus runs with
    --enable-ldw-opt=false). Safe: the weights value is identical, and the
    removed instruction's dependency edges are merged into its matmul."""
    def wkey(ldw):
        ap = ldw.ins[0]
        return (str(ap.memref), ap.offset, str(ap.ap), str(ap.dtype),
                str(ldw.is_transpose), str(getattr(ldw, "perf_mode", None)))

    removed = {}
    for fn in nc.m.functions:
        for blk in fn.blocks:
            insl = list(blk.instructions)
            # next PE instruction after each index
            next_pe = [None] * len(insl)
            nxt = None
            for i in range(len(insl) - 1, -1, -1):
                next_pe[i] = nxt
                if str(insl[i].engine) == "EngineType.PE":
                    nxt = insl[i]
            keep, last = [], None
            for i, ins in enumerate(insl):
                if str(ins.engine) != "EngineType.PE":
                    keep.append(ins)
                    continue
                if type(ins).__name__ != "InstLdweights":
                    keep.append(ins)
                    continue
                k = wkey(ins)
                mm = next_pe[i]
                if (k == last and mm is not None
                        and type(mm).__name__ == "InstMatmult"):
                    mm.merge_dependencies_from(ins)
                    removed[ins.name] = mm.name
                    continue
                last = k
                keep.append(ins)
            if len(keep) != len(insl):
                blk.instructions = keep
    if removed:
        for fn in nc.m.functions:
            for blk in fn.blocks:
                for ins in blk.instructions:
                    ins.remap_dependency_names(removed)


def _body(tc, q_tiles_d, k_tiles_d, out_tiles_d, score_dtype, repeat_c,
          reps_d=None, pv_dtype=FP16, swdge_cast=True, xbar_splits=4,
          scale_eng="alt", lookahead=2, load_mode="pe", s_order="dc",
          q_lazy=None, probe=None, timed_scope="c"):
    from contextlib import ExitStack

    nc = tc.nc
    reps_rv = None
    if reps_d is not None:
        regs = nc.alloc_registers("reps_regs")
        nc.regs_load(regs, reps_d[0:1, 0:1])
        reps_rv = nc.snap(regs, donate=True, min_val=0, max_val=64)
    with ExitStack() as ctx:
        persist = ctx.enter_context(tc.tile_pool(name="persist", bufs=1))
        work = ctx.enter_context(tc.tile_pool(name="work", bufs=4))
        small = ctx.enter_context(tc.tile_pool(name="small", bufs=4))
        load = ctx.enter_context(tc.tile_pool(name="load", bufs=4))
        n_ps_s = 6 if load_mode == "xbar" else 4
        ps_s = ctx.enter_context(
            tc.tile_pool(name="ps_s", bufs=n_ps_s, space="PSUM"))
        ps_pv = ctx.enter_context(tc.tile_pool(name="ps_pv", bufs=2, space="PSUM"))
        if load_mode in ("pe", "hybrid"):
            ps_tr = ctx.enter_context(
                tc.tile_pool(name="ps_tr", bufs=2, space="PSUM"))
            from concourse.masks import make_identity
            ident32 = persist.tile([P, P], FP32)
            make_identity(nc, ident32[:])
            ident16 = persist.tile([P, P], score_dtype)
            nc.vector.tensor_copy(ident16[:], ident32[:])

        # Natural fp16 copies (k16 doubles as the PV rhs) + transposes.
        # pe mode lays transposes out [d%128, dchunk, tile, 128] so the S rhs
        # slice (4 tiles, fixed dchunk) is contiguous; the XBAR's block order
        # forces [d%128, tile, dchunk, 128] (strided rhs) in xbar mode.
        k16 = persist.tile([P, NKT, D], pv_dtype)
        k_mode = "pe" if load_mode in ("pe", "hybrid") else "xbar"
        q_mode = "xbar" if load_mode in ("xbar", "hybrid") else "pe"
        if k_mode == "pe":
            kT = persist.tile([P, NDC, NKT, P], score_dtype)
            kT_s = lambda dc, lo, hi: kT[:, dc, lo:hi, :]
        else:
            kT = persist.tile([P, NKT, NDC, P], score_dtype)
            kT_s = lambda dc, lo, hi: kT[:, lo:hi, dc, :]
        # qT is only ever read as single [128,128] lhsT blocks, so it can use
        # the XBAR's forced t-major block order regardless of who writes it.
        qT = persist.tile([P, NQT, NDC, P], score_dtype)
        qT_s = lambda dc, i: qT[:, i, dc, :]

        # ---- Load phase, per 4-tile group: swdge casting DMA fp32->fp16,
        # then either an XBAR dma-transpose or fp16 PE transposes (PE idles
        # during load; keeps the DMA rings free for phase C's p transposes).
        # GPSIMD/Pool cannot touch PSUM, so PSUM->SBUF copies alternate
        # between ACT and DVE only.
        cast_engs = [nc.scalar, nc.vector, nc.gpsimd]
        # Load copies all go to DVE: any ACT-queue load copy delays the first
        # exps behind the slow cast->transpose chains (ACT SEQ is in-order).
        copy_engs = [nc.vector, nc.vector]
        copy_cnt = [0]

        # Warm the ACT exp table at t=0 so the first real exp doesn't pay
        # the 1.3us table load.
        warm = persist.tile([P, 1], FP32)
        nc.vector.memset(warm[:], 0.0)
        nc.scalar.activation(warm[:], warm[:], AF.Exp, bias=warm[:])

        def emit_cast(src_d, nat, g, qi):
            src = src_d[g * 4 : (g + 1) * 4].rearrange("t p d -> p t d")
            if swdge_cast:
                nc.gpsimd.dma_start(nat[:, g * 4 : (g + 1) * 4], src)
            else:
                gt = load.tile([P, 4, D], FP32, tag="ld")
                eng = nc.sync if g % 2 == 0 else nc.scalar
                eng.dma_start(gt[:], src)
                cast_engs[qi % 3].tensor_copy(nat[:, g * 4 : (g + 1) * 4], gt[:])

        def emit_tr(nat, dstT, g, mode, dc_major, engs=None, per_tile=False):
            """per_tile=True groups the 4 transposes+copy by q/k TILE instead
            of by d-chunk, so a consumer needing only the group's first tile
            (S(0) on Q g0) is gated on 1/4 of the chain."""
            if mode == "xbar":
                xq = nc.sync if g % 2 == 0 else nc.scalar
                xq.dma_start_transpose(
                    dstT[:, g * 4 : (g + 1) * 4], nat[:, g * 4 : (g + 1) * 4]
                )
                return
            engs = engs or copy_engs
            for a in range(NDC):
                ptr = ps_tr.tile([P, 4, P], score_dtype, tag="tr")
                for b in range(4):
                    j, dc = (a, b) if per_tile else (b, a)
                    nc.tensor.transpose(
                        ptr[:, b, :],
                        nat[:, g * 4 + j, dc * P : (dc + 1) * P],
                        ident16[:],
                    )
                eng = engs[copy_cnt[0] % 2]
                copy_cnt[0] += 1
                if per_tile:
                    dst = (dstT[:, :, g * 4 + a, :] if dc_major
                           else dstT[:, g * 4 + a, :, :])
                else:
                    dst = (dstT[:, dc, g * 4 : (g + 1) * 4, :] if dc_major
                           else dstT[:, g * 4 : (g + 1) * 4, dc, :])
                if eng is nc.scalar:
                    eng.copy(dst, ptr[:])
                else:
                    eng.tensor_copy(dst, ptr[:])

        def emit_load_group(src_d, nat, dstT, g, qi, mode, dc_major):
            emit_cast(src_d, nat, g, qi)
            emit_tr(nat, dstT, g, mode, dc_major)


        q16 = persist.tile([P, NQT, D], score_dtype)
        cbias = None
        if probe == "const_bias":
            cbias = persist.tile([P, 1], FP32)
            nc.vector.memset(cbias[:], -60.0)

        # Head: casts in need-order (K g0, Q g0, K g1..3); K g0 + Q g0
        # transposes now, K g1..3 transposes interleaved into tile 0's S
        # chunks (chunk c only needs K group c when tile 0 runs c-order),
        # so S(0) isn't queued on the PE behind transposes waiting for the
        # last K cast.
        head_engs = [nc.scalar, nc.vector]
        k_tr_emitted = [False, False, False, False]
        q_groups_emitted = [False, False, False, False]

        def emit_head():
            k_tr_emitted[:] = [False] * 4
            q_groups_emitted[:] = [False] * 4
            emit_cast(k_tiles_d, k16, 0, 0)
            emit_cast(q_tiles_d, q16, 0, 4)
            for g in range(1, 4):
                emit_cast(k_tiles_d, k16, g, g)
            emit_tr(k16, kT, 0, k_mode, k_mode == "pe", engs=head_engs)
            emit_tr(q16, qT, 0, q_mode, False, engs=head_engs)
            k_tr_emitted[0] = True
            q_groups_emitted[0] = True

        emit_head()

        def ensure_k_tr(g):
            if not k_tr_emitted[g]:
                emit_tr(k16, kT, g, k_mode, k_mode == "pe", engs=head_engs)
                k_tr_emitted[g] = True

        def ensure_q_group(i):
            # Lazy Q groups (1-3) land mid-phase-C where the PE is saturated
            # but the DMA rings are not: use the XBAR for their transposes
            # (q_lazy="xbar") instead of stealing PE cycles.
            g = i // 4
            if not q_groups_emitted[g]:
                mode = q_lazy if q_lazy else q_mode
                emit_load_group(q_tiles_d, q16, qT, g, 4 + g, mode, False)
                q_groups_emitted[g] = True

        # ---- Phase C: attention over q tiles, software-pipelined ----
        def emit_S(i, after=None, pre_chunk=None, pools=None):
            """S matmuls (4 separate PSUM chunk banks) + chunk maxes + negmax.
            s_order="dc" keeps lhsT constant across 4 consecutive matmuls
            (fewer LD_WEIGHTS) but interleaves the 4 PSUM accumulation
            groups; "c" completes each chunk before the next. pre_chunk(c)
            runs before chunk c's matmuls (forces c-order)."""
            chunks = [(pools[c] if pools else ps_s).tile(
                [P, 512], FP32, tag="s" if not pools or pools[c] is ps_s
                else "pv", name=f"s{i}_{c}") for c in range(NCH)]
            m4 = small.tile([P, NCH], FP32, tag="m4")
            negmax = small.tile([P, 1], FP32, tag="negmax")
            last_mm = None
            if s_order == "dc" and pre_chunk is None:
                loop = [(dc, c) for dc in range(NDC) for c in range(NCH)]
            else:
                loop = [(dc, c) for c in range(NCH) for dc in range(NDC)]
            # NOTE: don't pin the S matmul order to force same-lhsT adjacency
            # for the Ldweights dedup: full dc-outer serialization gets the
            # dedup to 463 loads but defers every chunk's accumulation stop
            # to the last dc group, delaying the max->exp chain by +23us sim
            # (~4x the ~6us of weight-load savings). The scheduler's chunk
            # interleave is load-bearing.
            seen = set()
            for dc, c in loop:
                if pre_chunk is not None and c not in seen:
                    seen.add(c)
                    pre_chunk(c)
                last_mm = nc.tensor.matmul(
                    chunks[c][:],
                    lhsT=qT_s(dc, i),
                    rhs=kT_s(dc, c * 4, (c + 1) * 4),
                    start=(dc == 0),
                    stop=(dc == NDC - 1),
                )
                if after is not None:
                    tile.add_dep_helper(
                        last_mm.ins, after.ins, False, "S-after-prev-PV"
                    )
                    after = None
                if dc == NDC - 1:
                    nc.vector.reduce_max(
                        m4[:, c : c + 1], chunks[c][:],
                        axis=mybir.AxisListType.X,
                    )
            nc.vector.reduce_max(
                negmax[:], m4[:], axis=mybir.AxisListType.X, negate=True
            )
            return chunks, negmax, last_mm

        def emit_E(i, chunks, negmax):
            """exp(S - max) per chunk -> p (fp16) + fused row-sums; each chunk
            is XBAR dma-transposed into pT as soon as its exp lands, so pT is
            complete shortly after the last exp instead of one full-tile
            transfer later."""
            p = work.tile([P, NCH, 512], pv_dtype, tag="p")
            pT = work.tile([P, NKT, P], pv_dtype, tag="pT")
            rs4 = small.tile([P, NCH], FP32, tag="rs4")
            rowsum = small.tile([P, 1], FP32, tag="rowsum")
            rinv = small.tile([P, 1], FP32, tag="rinv")
            w = NCH // xbar_splits
            for c in range(NCH):
                # probe=const_bias cuts the negmax->exp dependency (timing
                # probe only -- output is numerically wrong).
                bias = cbias[:] if probe == "const_bias" else negmax[:]
                nc.scalar.activation(
                    p[:, c, :], chunks[c][:], AF.Exp, bias=bias,
                    accum_out=rs4[:, c : c + 1],
                )
                if (c + 1) % w == 0:
                    s = c + 1 - w
                    nc.sync.dma_start_transpose(
                        pT[:, s * 4 : (c + 1) * 4, :], p[:, s : c + 1, :]
                    )
            nc.vector.reduce_sum(rowsum[:], rs4[:], axis=mybir.AxisListType.X)
            nc.vector.reciprocal(rinv[:], rowsum[:])
            return pT, rinv

        def emit_PV(i, pT, rinv, after=None):
            psum_o = ps_pv.tile([P, 512], FP32, tag="pv")
            for t in range(NKT):
                mm = nc.tensor.matmul(
                    psum_o[:],
                    lhsT=pT[:, t, :],
                    rhs=k16[:, t, :],
                    start=(t == 0),
                    stop=(t == NKT - 1),
                )
                if t == 0 and after is not None:
                    # Keep PV(i) behind S(i+1) on the PE queue so PV's work
                    # hides the max->exp->xbar latency of tile i+1.
                    tile.add_dep_helper(
                        mm.ins, after.ins, False, "pv-after-next-S"
                    )
            out_sb = work.tile([P, 512], FP32, tag="out_sb")
            # Pool can't read PSUM; alternate the scale between DVE and ACT.
            if scale_eng == "act" or (scale_eng == "alt" and i % 2):
                nc.scalar.mul(out_sb[:], psum_o[:], rinv[:])
            else:
                nc.vector.tensor_scalar_mul(out_sb[:], psum_o[:], rinv[:])
            oq = nc.sync if i >= NQT - 1 else nc.gpsimd
            oq.dma_start(out_tiles_d[i], out_sb[:])
            return mm

        def emit_C():
            if probe == "s_only":
                for i in range(NQT):
                    ensure_q_group(i)
                    pre = (lambda c: ensure_k_tr(c)) if i == 0 else None
                    emit_S(i, pre_chunk=pre)
                return
            la = lookahead
            state = {}
            for j in range(la):
                ensure_q_group(j)
                # Prefetch: emit trK(c+1) before chunk c's matmuls so the
                # next group's transposes overlap this chunk's compute.
                pre = (lambda c: ensure_k_tr(min(c + 1, 3))) if j == 0 else None
                # Tile 0 borrows the (still idle) PV banks for two of its
                # chunks so tiles 1-2 don't stall on S-bank reuse while the
                # pipeline fills.
                pools = [ps_pv, ps_s, ps_pv, ps_s] if j == 0 else None
                s_ps, s_nm, _ = emit_S(j, pre_chunk=pre, pools=pools)
                state[j] = emit_E(j, s_ps, s_nm)
            for i in range(NQT):
                pT, rinv = state.pop(i)
                after = None
                if i + la < NQT:
                    ensure_q_group(i + la)
                    s_ps, s_nm, after = emit_S(i + la)
                    state[i + la] = emit_E(i + la, s_ps, s_nm)
                emit_PV(i, pT, rinv, after=after)

        for _ in range(repeat_c):
            emit_C()

        if reps_rv is not None:
            with tc.For_i(0, reps_rv, 1):
                if timed_scope in ("full", "load"):
                    emit_head()
                if timed_scope == "load":
                    for g in range(1, 4):
                        ensure_k_tr(g)
                    for g in range(4):
                        ensure_q_group(g * 4)
                else:
                    emit_C()


_NC_CACHE = {}


def _get_nc(score_dtype=FP16, repeat_c=1):
    key = (str(score_dtype), repeat_c)
    if key not in _NC_CACHE:
        _NC_CACHE[key] = build(score_dtype, repeat_c)
    return _NC_CACHE[key]


def kernel(query: np.ndarray, key: np.ndarray) -> np.ndarray:
    query = np.asarray(query, dtype=np.float32)
    key = np.asarray(key, dtype=np.float32)
    assert query.shape == (B, NQ, D) and key.shape == (B, NK, D)
    nc = _get_nc()
    in_maps = [{"query": query[b], "key": key[b]} for b in range(B)]
    res = run_bass_kernel_spmd(nc, in_maps, list(range(B)))
    return np.stack([res.results[b]["out"] for b in range(B)], axis=0)



# revision 8
# speedup vs baseline: 6.9253x; 6.9253x over previous
"""TRN2 Bass kernel for nn_Attention_70257075028315.

reference:
    scores = einsum('bqd,bkd->bqk', query, key)       # B=8, Nq=Nk=2048, D=512
    probs  = softmax(scores, -1)
    out    = einsum('bqk,bkd->bqd', probs, key)

Sharding: batch b -> NeuronCore b (data parallel, fully local attention).

v2 design (per-core q/k: [2048, 512] fp32). Everything 16-bit on the PE,
p-transposes on the DMA XBAR, all loads cast in-flight by the software DGE:

  Load: per 4-tile group, a gpsimd (swdge) casting DMA moves fp32 DRAM ->
    fp16 SBUF natural layout (k16/q16, k16 doubles as the PV rhs); fp16 PE
    transposes (1 cyc/row; PE is idle during load) + DVE PSUM->SBUF copies
    build kT/qT [128(d%128), dchunk, tile, 128] (contiguous S rhs slices).
    Cast order K g0, Q g0, K g1-3; K group-g transposes are interleaved
    one chunk ahead into tile 0's S chunks (tile 0 runs c-order so chunk c
    needs only K group c), so S(0) isn't queued behind transposes waiting
    on the last cast; tile 0 borrows the idle PV PSUM banks for two chunks
    so tiles 1-2 don't stall on S-bank reuse while the pipeline fills. A
    dummy exp at t=0 preloads the ACT exp table (saves 1.3us on first exp).
  Phase C (per q-tile, software-pipelined with lookahead=2):
    S     = qT.T @ kT   fp16, 4 d-chunk-accumulated matmuls per 512-wide
            chunk, each chunk in its own PSUM bank; dc-outer order keeps
            lhsT constant over 4 consecutive matmuls (fewer weight loads)
    max   per chunk on DVE as each chunk completes; combined, negated
    p     = exp(S - max) fp16: one ACT pass per chunk PSUM -> SBUF with
            fused row-sum accum; 1/sum via DVE reciprocal
    pT    = per-chunk XBAR dma-transposes ([128,512]->[128,4,128] fp16,
            SP queue) fired as each exp lands -- no PE transposes and no
            PSUM round-trip for pT
    o     = pT.T @ k16  fp16, 16 kk-accumulated matmuls -> PSUM [128, 512]
    out   = o * (1/rowsum) on DVE/ACT (alternating), out-DMA on the Pool
            (swdge) queue to keep SP free for the p XBARs (last tile via
            sync HWDGE for a shorter drain).
  Emission per step i: S(i+2)+E(i+2), PV(i), with a PE-queue dep keeping
  PV(i) after S(i+2) so PV hides tile i+2's max->exp->xbar latency chain
  (~6us, needs the 2-tile lookahead). PSUM: 4 S banks + 2 PV + 2 load-tr.

Dtype: fp16 everywhere on the PE. HW-measured: 16-bit matmuls stream
1 col/cycle (same as f32r >=256 wide, 2x faster than fp32), but fp16
operands halve SBUF traffic and let kT/qT/pT/k16 fit in 44KB/partition.
Score error vs f32r only ~2x (fp16 keeps 10 mantissa bits); measured
output rel err 1.5e-3 (budget 2e-2). bf16 scores would be 8x worse and
fp8 fails the budget outright.
"""

import numpy as np

import concourse.bass as bass
import concourse.tile as tile
import concourse.mybir as mybir
from concourse import bacc
from concourse.bass_utils import run_bass_kernel_spmd

FP32 = mybir.dt.float32
FP32R = mybir.dt.float32r
FP16 = mybir.dt.float16
AF = mybir.ActivationFunctionType

B, NQ, NK, D = 8, 2048, 2048, 512
P = 128
NKT = NK // P   # 16 kk tiles
NQT = NQ // P   # 16 q tiles
NDC = D // P    # 4 d chunks
NCH = NK // 512  # 4 score chunks of 512


def build(score_dtype=FP16, repeat_c=1, timed=False, pv_dtype=FP16,
          swdge_cast=True, xbar_splits=4, scale_eng="alt", lookahead=2,
          load_mode="pe", s_order="dc", q_lazy=None, probe=None,
          dedup_ldw=True, kpv_bf16=False, timed_scope="c"):
    """timed=True adds an int32 [1,1] input "reps": phase C re-runs in a
    dynamic For_i loop `reps` more times (0 = just the normal kernel), so one
    NEFF can measure the phase-C slope against itself."""
    nc = bacc.Bacc("TRN2", target_bir_lowering=False, debug=False)
    q_d = nc.dram_tensor("query", [NQ, D], FP32, kind="ExternalInput").ap()
    k_d = nc.dram_tensor("key", [NK, D], FP32, kind="ExternalInput").ap()
    reps_d = None
    if timed:
        reps_d = nc.dram_tensor(
            "reps", [1, 1], mybir.dt.int32, kind="ExternalInput"
        ).ap()
    out_d = nc.dram_tensor("out", [NQ, D], FP32, kind="ExternalOutput").ap()

    q_tiles_d = q_d.rearrange("(t p) d -> t p d", p=P)
    k_tiles_d = k_d.rearrange("(t p) d -> t p d", p=P)
    out_tiles_d = out_d.rearrange("(t p) d -> t p d", p=P)

    with tile.TileContext(nc) as tc:
        _body(tc, q_tiles_d, k_tiles_d, out_tiles_d, score_dtype, repeat_c,
              reps_d, pv_dtype, swdge_cast, xbar_splits, scale_eng, lookahead,
              load_mode, s_order, q_lazy, probe, timed_scope)
    nc.compile()
    if dedup_ldw:
        # NOTE: do NOT reorder instructions here (e.g. regrouping S matmuls
        # by weights): compile() lowers waits to per-engine completion-count
        # semaphores, so same-engine reordering breaks wait semantics (the
        # race detector catches it). Removing a non-updating instruction and
        # merging its waits forward is the only safe post-compile transform.
        _dedup_ldweights(nc)
    return nc


def _dedup_ldweights(nc):
    """Drop InstLdweights that reload the exact weights already in the PE
    array (dc-outer S keeps lhsT constant across 4 consecutive matmuls, but
    the lowering emits one Ldweights per matmul and walrus runs with
    --enable-ldw-opt=false). Safe: the weights value is identical, and the
    removed instruction's dependency edges are merged into its matmul."""
    def wkey(ldw):
        ap = ldw.ins[0]
        return (str(ap.memref), ap.offset, str(ap.ap), str(ap.dtype),
                str(ldw.is_transpose), str(getattr(ldw, "perf_mode", None)))

    removed = {}
    for fn in nc.m.functions:
        for blk in fn.blocks:
            insl = list(blk.instructions)
            # next PE instruction after each index
            next_pe = [None] * len(insl)
            nxt = None
            for i in range(len(insl) - 1, -1, -1):
                next_pe[i] = nxt
                if str(insl[i].engine) == "EngineType.PE":
                    nxt = insl[i]
            keep, last = [], None
            for i, ins in enumerate(insl):
                if str(ins.engine) != "EngineType.PE":
                    keep.append(ins)
                    continue
                if type(ins).__name__ != "InstLdweights":
                    keep.append(ins)
                    continue
                k = wkey(ins)
                mm = next_pe[i]
                if (k == last and mm is not None
                        and type(mm).__name__ == "InstMatmult"):
                    mm.merge_dependencies_from(ins)
                    removed[ins.name] = mm.name
                    continue
                last = k
                keep.append(ins)
            if len(keep) != len(insl):
                blk.instructions = keep
    if removed:
        for fn in nc.m.functions:
            for blk in fn.blocks:
                for ins in blk.instructions:
                    ins.remap_dependency_names(removed)


def _body(tc, q_tiles_d, k_tiles_d, out_tiles_d, score_dtype, repeat_c,
          reps_d=None, pv_dtype=FP16, swdge_cast=True, xbar_splits=4,
          scale_eng="alt", lookahead=2, load_mode="pe", s_order="dc",
          q_lazy=None, probe=None, timed_scope="c"):
    from contextlib import ExitStack

    nc = tc.nc
    reps_rv = None
    if reps_d is not None:
        regs = nc.alloc_registers("reps_regs")
        nc.regs_load(regs, reps_d[0:1, 0:1])
        reps_rv = nc.snap(regs, donate=True, min_val=0, max_val=64)
    with ExitStack() as ctx:
        persist = ctx.enter_context(tc.tile_pool(name="persist", bufs=1))
        work = ctx.enter_context(tc.tile_pool(name="work", bufs=4))
        small = ctx.enter_context(tc.tile_pool(name="small", bufs=4))
        load = ctx.enter_context(tc.tile_pool(name="load", bufs=4))
        n_ps_s = 6 if load_mode == "xbar" else 4
        ps_s = ctx.enter_context(
            tc.tile_pool(name="ps_s", bufs=n_ps_s, space="PSUM"))
        ps_pv = ctx.enter_context(tc.tile_pool(name="ps_pv", bufs=2, space="PSUM"))
        if load_mode in ("pe", "hybrid"):
            ps_tr = ctx.enter_context(
                tc.tile_pool(name="ps_tr", bufs=2, space="PSUM"))
            from concourse.masks import make_identity
            ident32 = persist.tile([P, P], FP32)
            make_identity(nc, ident32[:])
            ident16 = persist.tile([P, P], score_dtype)
            nc.vector.tensor_copy(ident16[:], ident32[:])

        # Natural fp16 copies (k16 doubles as the PV rhs) + transposes.
        # pe mode lays transposes out [d%128, dchunk, tile, 128] so the S rhs
        # slice (4 tiles, fixed dchunk) is contiguous; the XBAR's block order
        # forces [d%128, tile, dchunk, 128] (strided rhs) in xbar mode.
        k16 = persist.tile([P, NKT, D], pv_dtype)
        k_mode = "pe" if load_mode in ("pe", "hybrid") else "xbar"
        q_mode = "xbar" if load_mode in ("xbar", "hybrid") else "pe"
        if k_mode == "pe":
            kT = persist.tile([P, NDC, NKT, P], score_dtype)
            kT_s = lambda dc, lo, hi: kT[:, dc, lo:hi, :]
        else:
            kT = persist.tile([P, NKT, NDC, P], score_dtype)
            kT_s = lambda dc, lo, hi: kT[:, lo:hi, dc, :]
        # qT is only ever read as single [128,128] lhsT blocks, so it can use
        # the XBAR's forced t-major block order regardless of who writes it.
        qT = persist.tile([P, NQT, NDC, P], score_dtype)
        qT_s = lambda dc, i: qT[:, i, dc, :]

        # ---- Load phase, per 4-tile group: swdge casting DMA fp32->fp16,
        # then either an XBAR dma-transpose or fp16 PE transposes (PE idles
        # during load; keeps the DMA rings free for phase C's p transposes).
        # GPSIMD/Pool cannot touch PSUM, so PSUM->SBUF copies alternate
        # between ACT and DVE only.
        cast_engs = [nc.scalar, nc.vector, nc.gpsimd]
        # Load copies all go to DVE: any ACT-queue load copy delays the first
        # exps behind the slow cast->transpose chains (ACT SEQ is in-order).
        copy_engs = [nc.vector, nc.vector]
        copy_cnt = [0]

        # Warm the ACT exp table at t=0 so the first real exp doesn't pay
        # the 1.3us table load.
        warm = persist.tile([P, 1], FP32)
        nc.vector.memset(warm[:], 0.0)
        nc.scalar.activation(warm[:], warm[:], AF.Exp, bias=warm[:])

        def emit_cast(src_d, nat, g, qi):
            src = src_d[g * 4 : (g + 1) * 4].rearrange("t p d -> p t d")
            if swdge_cast:
                nc.gpsimd.dma_start(nat[:, g * 4 : (g + 1) * 4], src)
            else:
                gt = load.tile([P, 4, D], FP32, tag="ld")
                eng = nc.sync if g % 2 == 0 else nc.scalar
                eng.dma_start(gt[:], src)
                cast_engs[qi % 3].tensor_copy(nat[:, g * 4 : (g + 1) * 4], gt[:])

        def emit_tr(nat, dstT, g, mode, dc_major, engs=None, per_tile=False):
            """per_tile=True groups the 4 transposes+copy by q/k TILE instead
            of by d-chunk, so a consumer needing only the group's first tile
            (S(0) on Q g0) is gated on 1/4 of the chain."""
            if mode == "xbar":
                xq = nc.sync if g % 2 == 0 else nc.scalar
                xq.dma_start_transpose(
                    dstT[:, g * 4 : (g + 1) * 4], nat[:, g * 4 : (g + 1) * 4]
                )
                return
            engs = engs or copy_engs
            for a in range(NDC):
                ptr = ps_tr.tile([P, 4, P], score_dtype, tag="tr")
                for b in range(4):
                    j, dc = (a, b) if per_tile else (b, a)
                    nc.tensor.transpose(
                        ptr[:, b, :],
                        nat[:, g * 4 + j, dc * P : (dc + 1) * P],
                        ident16[:],
                    )
                eng = engs[copy_cnt[0] % 2]
                copy_cnt[0] += 1
                if per_tile:
                    dst = (dstT[:, :, g * 4 + a, :] if dc_major
                           else dstT[:, g * 4 + a, :, :])
                else:
                    dst = (dstT[:, dc, g * 4 : (g + 1) * 4, :] if dc_major
                           else dstT[:, g * 4 : (g + 1) * 4, dc, :])
                if eng is nc.scalar:
                    eng.copy(dst, ptr[:])
                else:
                    eng.tensor_copy(dst, ptr[:])

        def emit_load_group(src_d, nat, dstT, g, qi, mode, dc_major):
            emit_cast(src_d, nat, g, qi)
            emit_tr(nat, dstT, g, mode, dc_major)


        q16 = persist.tile([P, NQT, D], score_dtype)
        cbias = None
        if probe == "const_bias":
            cbias = persist.tile([P, 1], FP32)
            nc.vector.memset(cbias[:], -60.0)

        # Head: casts in need-order (K g0, Q g0, K g1..3); K g0 + Q g0
        # transposes now, K g1..3 transposes interleaved into tile 0's S
        # chunks (chunk c only needs K group c when tile 0 runs c-order),
        # so S(0) isn't queued on the PE behind transposes waiting for the
        # last K cast.
        head_engs = [nc.scalar, nc.vector]
        k_tr_emitted = [False, False, False, False]
        q_groups_emitted = [False, False, False, False]

        def emit_head():
            k_tr_emitted[:] = [False] * 4
            q_groups_emitted[:] = [False] * 4
            emit_cast(k_tiles_d, k16, 0, 0)
            emit_cast(q_tiles_d, q16, 0, 4)
            for g in range(1, 4):
                emit_cast(k_tiles_d, k16, g, g)
            emit_tr(k16, kT, 0, k_mode, k_mode == "pe", engs=head_engs)
            emit_tr(q16, qT, 0, q_mode, False, engs=head_engs)
            k_tr_emitted[0] = True
            q_groups_emitted[0] = True

        emit_head()

        def ensure_k_tr(g):
            if not k_tr_emitted[g]:
                emit_tr(k16, kT, g, k_mode, k_mode == "pe", engs=head_engs)
                k_tr_emitted[g] = True

        def ensure_q_group(i):
            # Lazy Q groups (1-3) land mid-phase-C where the PE is saturated
            # but the DMA rings are not: use the XBAR for their transposes
            # (q_lazy="xbar") instead of stealing PE cycles.
            g = i // 4
            if not q_groups_emitted[g]:
                mode = q_lazy if q_lazy else q_mode
                emit_load_group(q_tiles_d, q16, qT, g, 4 + g, mode, False)
                q_groups_emitted[g] = True

        # ---- Phase C: attention over q tiles, software-pipelined ----
        def emit_S(i, after=None, pre_chunk=None, pools=None):
            """S matmuls (4 separate PSUM chunk banks) + chunk maxes + negmax.
            s_order="dc" keeps lhsT constant across 4 consecutive matmuls
            (fewer LD_WEIGHTS) but interleaves the 4 PSUM accumulation
            groups; "c" completes each chunk before the next. pre_chunk(c)
            runs before chunk c's matmuls (forces c-order)."""
            chunks = [(pools[c] if pools else ps_s).tile(
                [P, 512], FP32, tag="s" if not pools or pools[c] is ps_s
                else "pv", name=f"s{i}_{c}") for c in range(NCH)]
            m4 = small.tile([P, NCH], FP32, tag="m4")
            negmax = small.tile([P, 1], FP32, tag="negmax")
            last_mm = None
            if s_order == "dc" and pre_chunk is None:
                loop = [(dc, c) for dc in range(NDC) for c in range(NCH)]
            else:
                loop = [(dc, c) for c in range(NCH) for dc in range(NDC)]
            # NOTE: don't pin the S matmul order to force same-lhsT adjacency
            # for the Ldweights dedup: full dc-outer serialization gets the
            # dedup to 463 loads but defers every chunk's accumulation stop
            # to the last dc group, delaying the max->exp chain by +23us sim
            # (~4x the ~6us of weight-load savings). The scheduler's chunk
            # interleave is load-bearing.
            seen = set()
            for dc, c in loop:
                if pre_chunk is not None and c not in seen:
                    seen.add(c)
                    pre_chunk(c)
                last_mm = nc.tensor.matmul(
                    chunks[c][:],
                    lhsT=qT_s(dc, i),
                    rhs=kT_s(dc, c * 4, (c + 1) * 4),
                    start=(dc == 0),
                    stop=(dc == NDC - 1),
                )
                if after is not None:
                    tile.add_dep_helper(
                        last_mm.ins, after.ins, False, "S-after-prev-PV"
                    )
                    after = None
                if dc == NDC - 1:
                    nc.vector.reduce_max(
                        m4[:, c : c + 1], chunks[c][:],
                        axis=mybir.AxisListType.X,
                    )
            nc.vector.reduce_max(
                negmax[:], m4[:], axis=mybir.AxisListType.X, negate=True
            )
            return chunks, negmax, last_mm

        def emit_E(i, chunks, negmax):
            """exp(S - max) per chunk -> p (fp16) + fused row-sums; each chunk
            is XBAR dma-transposed into pT as soon as its exp lands, so pT is
            complete shortly after the last exp instead of one full-tile
            transfer later."""
            p = work.tile([P, NCH, 512], pv_dtype, tag="p")
            pT = work.tile([P, NKT, P], pv_dtype, tag="pT")
            rs4 = small.tile([P, NCH], FP32, tag="rs4")
            rowsum = small.tile([P, 1], FP32, tag="rowsum")
            rinv = small.tile([P, 1], FP32, tag="rinv")
            w = NCH // xbar_splits
            for c in range(NCH):
                # probe=const_bias cuts the negmax->exp dependency (timing
                # probe only -- output is numerically wrong).
                bias = cbias[:] if probe == "const_bias" else negmax[:]
                nc.scalar.activation(
                    p[:, c, :], chunks[c][:], AF.Exp, bias=bias,
                    accum_out=rs4[:, c : c + 1],
                )
                if (c + 1) % w == 0:
                    s = c + 1 - w
                    nc.sync.dma_start_transpose(
                        pT[:, s * 4 : (c + 1) * 4, :], p[:, s : c + 1, :]
                    )
            nc.vector.reduce_sum(rowsum[:], rs4[:], axis=mybir.AxisListType.X)
            nc.vector.reciprocal(rinv[:], rowsum[:])
            return pT, rinv

        def emit_PV(i, pT, rinv, after=None):
            psum_o = ps_pv.tile([P, 512], FP32, tag="pv")
            for t in range(NKT):
                mm = nc.tensor.matmul(
                    psum_o[:],
                    lhsT=pT[:, t, :],
                    rhs=k16[:, t, :],
                    start=(t == 0),
                    stop=(t == NKT - 1),
                )
                if t == 0 and after is not None:
                    # Keep PV(i) behind S(i+1) on the PE queue so PV's work
                    # hides the max->exp->xbar latency of tile i+1.
                    tile.add_dep_helper(
                        mm.ins, after.ins, False, "pv-after-next-S"
                    )
            out_sb = work.tile([P, 512], FP32, tag="out_sb")
            # Pool can't read PSUM; alternate the scale between DVE and ACT.
            if scale_eng == "act" or (scale_eng == "alt" and i % 2):
                nc.scalar.mul(out_sb[:], psum_o[:], rinv[:])
            else:
                nc.vector.tensor_scalar_mul(out_sb[:], psum_o[:], rinv[:])
            oq = nc.sync if i >= NQT - 1 else nc.gpsimd
            oq.dma_start(out_tiles_d[i], out_sb[:])
            return mm

        def emit_C():
            if probe == "s_only":
                for i in range(NQT):
                    ensure_q_group(i)
                    pre = (lambda c: ensure_k_tr(c)) if i == 0 else None
                    emit_S(i, pre_chunk=pre)
                return
            if probe == "mm_only":
                # Pure-PE probe: S matmuls + PV matmuls with a constant lhsT
                # (no softmax dependency). Measures the PE-side floor incl.
                # LDWEIGHTS behavior. Output is numerically wrong.
                for i in range(NQT):
                    ensure_q_group(i)
                    pre = (lambda c: ensure_k_tr(c)) if i == 0 else None
                    emit_S(i, pre_chunk=pre)
                    psum_o = ps_pv.tile([P, 512], FP32, tag="pv")
                    for t in range(NKT):
                        nc.tensor.matmul(
                            psum_o[:], lhsT=qT_s(0, i), rhs=k16[:, t, :],
                            start=(t == 0), stop=(t == NKT - 1),
                        )
                return
            la = lookahead
            state = {}
            for j in range(la):
                ensure_q_group(j)
                # Prefetch: emit trK(c+1) before chunk c's matmuls so the
                # next group's transposes overlap this chunk's compute.
                pre = (lambda c: ensure_k_tr(min(c + 1, 3))) if j == 0 else None
                # Tile 0 borrows the (still idle) PV banks for two of its
                # chunks so tiles 1-2 don't stall on S-bank reuse while the
                # pipeline fills.
                pools = [ps_pv, ps_s, ps_pv, ps_s] if j == 0 else None
                s_ps, s_nm, _ = emit_S(j, pre_chunk=pre, pools=pools)
                state[j] = emit_E(j, s_ps, s_nm)
            for i in range(NQT):
                pT, rinv = state.pop(i)
                after = None
                if i + la < NQT:
                    ensure_q_group(i + la)
                    s_ps, s_nm, after = emit_S(i + la)
                    state[i + la] = emit_E(i + la, s_ps, s_nm)
                emit_PV(i, pT, rinv, after=after)

        for _ in range(repeat_c):
            emit_C()

        if reps_rv is not None:
            with tc.For_i(0, reps_rv, 1):
                if timed_scope in ("full", "load"):
                    emit_head()
                if timed_scope == "load":
                    for g in range(1, 4):
                        ensure_k_tr(g)
                    for g in range(4):
                        ensure_q_group(g * 4)
                else:
                    emit_C()


_NC_CACHE = {}


def _get_nc(score_dtype=FP16, repeat_c=1):
    key = (str(score_dtype), repeat_c)
    if key not in _NC_CACHE:
        _NC_CACHE[key] = build(score_dtype, repeat_c)
    return _NC_CACHE[key]


def kernel(query: np.ndarray, key: np.ndarray) -> np.ndarray:
    query = np.asarray(query, dtype=np.float32)
    key = np.asarray(key, dtype=np.float32)
    assert query.shape == (B, NQ, D) and key.shape == (B, NK, D)
    nc = _get_nc()
    in_maps = [{"query": query[b], "key": key[b]} for b in range(B)]
    res = run_bass_kernel_spmd(nc, in_maps, list(range(B)))
    return np.stack([res.results[b]["out"] for b in range(B)], axis=0)

